# revision 1
# baseline (speedup 1.0000x reference)
"""3-layer GCN encoder (GCNConv x4, layers 3+4 fused) on 8 Trainium2 NeuronCores.

Strategy (graph/data parallel, matches the edge-cut sharding hint):
  - Nodes are partitioned contiguously across the 8 cores (NLOC = N/8 per core).
  - Each layer: local transform H = h @ W (PE, fp32), rows scaled by dinv[node],
    cast to bf16, packed two-nodes-per-256B-row into a local table slice, then
    AllGather -> full table in each core's HBM.
  - Aggregation: per 128-dst-node window, dma_gather fetches the (src-pair) rows
    for all in-edges (dst-grouped, padded to x128); a one-hot matrix S built on
    DVE (iota-256 is_equal dst_rel + 128*parity) turns segment-sum into PE
    matmuls accumulating in PSUM (node-major [128,64] f32).
  - Epilogue: x dinv[dst], + bias, ReLU (layers 1,2); final layer writes
    [NLOC, 64] = [mu | logstd] to DRAM.
  - Pair packing keeps gather indices = src>>1 < 32768 (int16 limit), halves
    the exchanged table bytes, and the parity select folds into the one-hot.

Self-contained: only needs numpy/ml_dtypes/concourse (container-installed).
"""

import os
import sys

if "/opt/trn_rl_repo" not in sys.path:
    sys.path.insert(0, "/opt/trn_rl_repo")

import numpy as np
import ml_dtypes

import concourse.bass as bass
import concourse.bacc as bacc
import concourse.mybir as mybir
import concourse.tile as tile
from concourse.bass_utils import run_bass_kernel_spmd

BF16 = ml_dtypes.bfloat16
F32 = mybir.dt.float32
BF = mybir.dt.bfloat16
I16 = mybir.dt.int16

N_CORES = 8

_cache = {}
_last = {}


def last_run(trace=False, **kw):
    """Re-run the last compiled kernel/in_maps (optionally with NTFF tracing)."""
    if "nc" not in _last:
        return None
    return run_bass_kernel_spmd(_last["nc"], _last["in_maps"],
                                core_ids=list(range(N_CORES)), trace=trace, **kw)


def _balance(deg, N, NLOC, W):
    """Deal degree-sorted nodes round-robin into windows, per core."""
    lpos = np.empty(N, np.int64)
    caps = np.full(W, 128, np.int64)
    caps[W - 1] = NLOC - 128 * (W - 1)
    for c in range(N_CORES):
        dl = deg[c * NLOC:(c + 1) * NLOC]
        order_ = np.argsort(-dl, kind="stable")
        fill = np.zeros(W, np.int64)
        wi = 0
        pos = np.empty(NLOC, np.int64)
        for i in range(NLOC):
            while fill[wi % W] >= caps[wi % W]:
                wi += 1
            ww = wi % W
            pos[order_[i]] = ww * 128 + fill[ww]
            fill[ww] += 1
            wi += 1
        lpos[c * NLOC:(c + 1) * NLOC] = pos
    return lpos


def _prep_edges(src, dst, N, NLOC, W, lpos=None):
    """Group in-edges by (dst core, dst window); pad each window to x128 slots
    uniformly across cores. Returns per-core int16 gather indices / bf16 dsel
    tiles plus per-window padded counts."""
    Wp = W + (W & 1)          # windows padded to even (pair = adjacent windows)
    TA = (Wp // 2 + 1) // 2   # chunk-A t-range (src windows [0, 2*TA))
    TB = Wp // 2 - TA         # chunk-B t-range
    RA, RB = 128 * TA, 128 * TB
    if lpos is None:
        lpos = np.arange(len(np.empty(0)))  # placeholder
        lpos = np.arange(N, dtype=np.int64) % NLOC
    core = dst // NLOC
    local = lpos[dst]
    w = local >> 7
    rel = local & 127
    # source -> (core, partition slot, window) -> chunk / row / parity
    sc = src // NLOC
    sl = lpos[src]
    sp = sl & 127
    sw = sl >> 7
    st = sw >> 1
    spar = sw & 1
    isB = st >= TA
    srow = np.where(isB, sc * RB + sp * TB + (st - TA),
                    sc * RA + sp * TA + st)

    # group edges by (core, dst-window, chunk)
    key = (core * W + w) * 2 + isB
    order = np.argsort(key, kind="stable")
    ksort = key[order]
    counts = np.bincount(key, minlength=N_CORES * W * 2).reshape(N_CORES, W, 2)
    P_As = np.maximum((counts[:, :, 0].max(0) + 127) // 128 * 128, 128)
    if TB > 0:
        P_Bs = np.maximum((counts[:, :, 1].max(0) + 127) // 128 * 128, 128)
    else:
        P_Bs = np.zeros(W, np.int64)
    PA_tot = int(P_As.sum())
    # slot layout: [all A segments by window][all B segments by window]
    cumA = np.concatenate([[0], np.cumsum(P_As)])
    cumB = np.concatenate([[0], np.cumsum(P_Bs)]) + PA_tot
    P_tot = int(PA_tot + P_Bs.sum())
    gstart = np.concatenate([[0], np.cumsum(counts.reshape(-1))])
    pos_in_group = np.arange(len(ksort)) - gstart[ksort]
    w_of = (ksort >> 1) % W
    c_of = (ksort >> 1) // W
    b_of = ksort & 1
    slot = np.where(b_of == 1, cumB[w_of], cumA[w_of]) + pos_in_group

    idx_arr = np.zeros((N_CORES, P_tot), np.int16)
    dsel_arr = np.full((N_CORES, P_tot), 300.0, np.float32)
    idx_arr[c_of, slot] = srow[order].astype(np.int16)
    dsel_arr[c_of, slot] = rel[order] + 128.0 * spar[order]

    idx_tiles = []
    dsel_tiles = []
    for c in range(N_CORES):
        idx16 = np.ascontiguousarray(idx_arr[c].reshape(P_tot // 16, 16).T)
        idx_tiles.append(np.ascontiguousarray(np.tile(idx16, (8, 1))))
        dsel_tiles.append(
            np.ascontiguousarray(dsel_arr[c].reshape(P_tot // 128, 128).T)
        )
    return idx_tiles, dsel_tiles, (list(map(int, P_As)), list(map(int, P_Bs))), P_tot


def _build(N, NLOC, W, P_ABs):
    """Build the 8-core SPMD Bass program. Returns compiled nc."""
    P_As, P_Bs = [list(map(int, p)) for p in P_ABs]
    PA_tot = sum(P_As)
    P_tot = PA_tot + sum(P_Bs)
    NTA = [p // 128 for p in P_As]
    NTB = [p // 128 for p in P_Bs]
    NT_MAX = max(NTA + NTB)
    cumA = np.concatenate([[0], np.cumsum(P_As)]).astype(int)
    cumB = (np.concatenate([[0], np.cumsum(P_Bs)]) + PA_tot).astype(int)
    GRP = int(os.environ.get("K_GRP", "1"))  # windows per gather instruction

    solo = os.environ.get("K_SOLO", "0") == "1"
    nc = bacc.Bacc("TRN2", target_bir_lowering=False, debug=False,
                   num_devices=1 if solo else N_CORES)

    xT_d = nc.dram_tensor("xT", (128, NLOC), F32, kind="ExternalInput")
    idxs_d = nc.dram_tensor("idxs", (128, P_tot // 16), I16, kind="ExternalInput")
    dsel_d = nc.dram_tensor("dsel", (128, P_tot // 128), F32, kind="ExternalInput")
    degp_d = nc.dram_tensor("degp", (128, W), F32, kind="ExternalInput")
    W1_d = nc.dram_tensor("W1", (128, 64), F32, kind="ExternalInput")
    W2_d = nc.dram_tensor("W2", (64, 64), F32, kind="ExternalInput")
    W34_d = nc.dram_tensor("W34", (64, 64), F32, kind="ExternalInput")
    b1_d = nc.dram_tensor("b1b", (128, 64), F32, kind="ExternalInput")
    b2_d = nc.dram_tensor("b2b", (128, 64), F32, kind="ExternalInput")
    b34_d = nc.dram_tensor("b34b", (128, 64), F32, kind="ExternalInput")
    iota_d = nc.dram_tensor("iota", (128, 256), BF, kind="ExternalInput")
    id128_d = nc.dram_tensor("id128", (128, 128), F32, kind="ExternalInput")
    id64_d = nc.dram_tensor("id64", (64, 64), BF, kind="ExternalInput")
    out_d = nc.dram_tensor("out34", (NLOC, 64), F32, kind="ExternalOutput")

    Wp = W + (W & 1)
    TA = (Wp // 2 + 1) // 2
    TB = Wp // 2 - TA
    RA, RB = 128 * TA, 128 * TB
    tablA = [nc.dram_tensor(f"tablA{l}", (RA, 128), BF, kind="Internal")
             for l in range(3)]
    split = TB > 0
    tablB = [nc.dram_tensor(f"tablB{l}", (RB, 128), BF, kind="Internal")
             for l in range(3)] if split else None
    tabfA = [nc.dram_tensor(f"tabfA{l}", (N_CORES * RA, 128), BF, kind="Internal",
                            addr_space="Shared") for l in range(3)]
    tabfB = [nc.dram_tensor(f"tabfB{l}", (N_CORES * RB, 128), BF, kind="Internal",
                            addr_space="Shared") for l in range(3)] if split else None

    AG = mybir.AluOpType
    RG = [list(range(N_CORES))]

    with tile.TileContext(nc) as tc:
        with (
            tc.tile_pool(name="const", bufs=1) as const,
            tc.tile_pool(name="big", bufs=1) as big,
            tc.tile_pool(name="tt", bufs=int(os.environ.get("K_TT","2"))) as ttp,
            tc.tile_pool(name="work", bufs=int(os.environ.get("K_WK","8"))) as work,
            tc.tile_pool(name="gp", bufs=int(os.environ.get("K_GP","8"))) as gp,
            tc.tile_pool(name="sp", bufs=int(os.environ.get("K_SB","8"))) as sp,
            tc.tile_pool(name="psT", bufs=2, space="PSUM") as psT,
            tc.tile_pool(name="psR", bufs=2, space="PSUM") as psR,
            tc.tile_pool(name="psA", bufs=int(os.environ.get("K_PSA","4")), space="PSUM") as psA,
        ):
            # ---- constant loads ----
            def cload(dram, shape, dt, tag):
                t = const.tile(shape, dt, tag=tag)
                nc.sync.dma_start(t[:], dram[:])
                return t

            idxs = cload(idxs_d, [128, P_tot // 16], I16, "idxs")
            dsel = cload(dsel_d, [128, P_tot // 128], F32, "dsel")
            iota = cload(iota_d, [128, 256], BF, "iota")
            id128 = cload(id128_d, [128, 128], F32, "id128")
            id64 = cload(id64_d, [64, 64], BF, "id64")
            W1t = cload(W1_d, [128, 64], F32, "W1t")
            W2t = cload(W2_d, [64, 64], F32, "W2t")
            W34t = cload(W34_d, [64, 64], F32, "W34t")
            b1t = cload(b1_d, [128, 64], F32, "b1t")
            b2t = cload(b2_d, [128, 64], F32, "b2t")
            b34t = cload(b34_d, [128, 64], F32, "b34t")

            NCH = (NLOC + 511) // 512
            hT2 = [big.tile([64, 512], F32, name=f"hT2_{j}", tag=f"hT2_{j}")
                   for j in range(NCH)]
            hT3 = [big.tile([64, 512], F32, name=f"hT3_{j}", tag=f"hT3_{j}")
                   for j in range(NCH)]

            degp = const.tile([128, W], F32, tag="degp")
            nc.sync.dma_start(degp[:], degp_d[:])
            sqp = const.tile([128, W], F32, tag="sqp")
            nc.scalar.activation(sqp[:], degp[:], mybir.ActivationFunctionType.Sqrt)
            dinvp = const.tile([128, W], F32, tag="dinvp")
            nc.vector.reciprocal(dinvp[:], sqp[:])

            CP = int(os.environ.get("K_CP", "0"))

            def nw_cols(nw):
                return 64

            def transform(l, hT, K, Wt):
                """T^T = W^T @ hT, bf16 [64, NLOC] (dinv applied in build_table)."""
                TT = ttp.tile([64, NLOC], BF, tag="TT")
                for c0 in range(0, NLOC, 512):
                    cn = min(512, NLOC - c0)
                    if hT is None:
                        xc = work.tile([128, 512], F32, tag="xc")
                        nc.sync.dma_start(xc[:, :cn], xT_d[:, c0:c0 + cn])
                        rhs = xc[:K, :cn]
                    else:
                        rhs = hT[c0 // 512][:K, :cn]
                    ps = psT.tile([64, 512], F32, tag="psT")
                    nc.tensor.matmul(ps[:, :cn], Wt[:K, :], rhs,
                                     start=True, stop=True)
                    if CP & 2:
                        nc.vector.tensor_copy(TT[:, c0:c0 + cn], ps[:, :cn])
                    else:
                        nc.scalar.copy(TT[:, c0:c0 + cn], ps[:, :cn])
                return TT

            def build_table(l, TT):
                stage = ttp.tile([128, Wp * 64], BF, tag="stage")
                if W != Wp:
                    nc.vector.memset(stage[:, W * 64:], 0.0)
                for w in range(W):
                    c0 = 128 * w
                    nw = min(128, NLOC - c0)
                    ptt = psR.tile([128, 128], BF, tag="ptr")
                    pt = ptt[:, :64]
                    nc.tensor.transpose(pt[:nw, :], TT[:, c0:c0 + nw], id64[:, :])
                    if nw < 128:
                        nc.vector.memset(stage[:, 64 * w:64 * (w + 1)], 0.0)
                    nc.scalar.activation(
                        stage[:nw, 64 * w:64 * w + 64], pt[:nw, :],
                        mybir.ActivationFunctionType.Copy,
                        scale=dinvp[:nw, w:w + 1])
                CA = 2 * TA * 64
                nc.sync.dma_start(
                    tablA[l][:].rearrange("(p r) e -> p (r e)", p=128),
                    stage[:, :CA])
                if split:
                    nc.sync.dma_start(
                        tablB[l][:].rearrange("(p r) e -> p (r e)", p=128),
                        stage[:, CA:])
                if not solo:
                    nc.gpsimd.collective_compute(
                        "AllGather", AG.bypass, replica_groups=RG,
                        ins=[tablA[l][:].opt()], outs=[tabfA[l][:].opt()])
                    if split:
                        nc.gpsimd.collective_compute(
                            "AllGather", AG.bypass, replica_groups=RG,
                            ins=[tablB[l][:].opt()], outs=[tabfB[l][:].opt()])

            def aggregate(l, bias_t, relu, hT_next):
                partial = big.tile([128, W * 64], F32, name=f"partA{l}",
                                   tag="partA")

                def seg_pass(is_b):
                    last = is_b or not split
                    tabsrc = (tabfB if is_b else tabfA)[l]
                    nts = NTB if is_b else NTA
                    cums = cumB if is_b else cumA
                    pws = P_Bs if is_b else P_As
                    for w0 in range(0, W, GRP):
                        wn = min(GRP, W - w0)
                        ntg = sum(nts[w0:w0 + wn])
                        pg = int(cums[w0 + wn] - cums[w0])
                        soff = int(cums[w0]) // 16
                        g = gp.tile([128, GRP * NT_MAX, 128], BF, tag="g")
                        nc.gpsimd.dma_gather(
                            g[:, :ntg, :], tabsrc[:],
                            idxs[:, soff: soff + pg // 16],
                            pg, pg, 128, single_packet=False)
                        tb = 0
                        for w in range(w0, w0 + wn):
                            c0 = 128 * w
                            nw = min(128, NLOC - c0)
                            nt = nts[w]
                            toff = int(cums[w]) // 128
                            ps = psA.tile([128, 64], F32, tag="psA")
                            for t in range(nt):
                                S = sp.tile([128, 256], BF, tag="S")
                                nc.vector.tensor_scalar(
                                    out=S[:], in0=iota[:],
                                    scalar1=dsel[:, toff + t: toff + t + 1],
                                    scalar2=None, op0=AG.is_equal)
                                nc.tensor.matmul(
                                    ps[:], S[:, 0:128], g[:, tb + t, 0:64],
                                    start=(t == 0), stop=False)
                                nc.tensor.matmul(
                                    ps[:], S[:, 128:256], g[:, tb + t, 64:128],
                                    start=False, stop=(t == nt - 1))
                            tb += nt
                            if not last:
                                if CP & 1:
                                    nc.vector.tensor_copy(
                                        partial[:, 64 * w:64 * w + 64], ps[:])
                                else:
                                    nc.scalar.copy(
                                        partial[:, 64 * w:64 * w + 64], ps[:])
                                continue
                            # last pass epilogue: combine, scale, bias, relu
                            hw_ = work.tile([128, 64], F32, tag="hw")
                            if split:
                                nc.vector.tensor_tensor(
                                    out=hw_[:], in0=ps[:],
                                    in1=partial[:, 64 * w:64 * w + 64],
                                    op=AG.add)
                            else:
                                nc.scalar.copy(hw_[:], ps[:])
                            if CP & 8:
                                nc.vector.tensor_scalar(
                                    out=hw_[:], in0=hw_[:],
                                    scalar1=dinvp[:, w:w + 1],
                                    scalar2=None, op0=AG.mult)
                            else:
                                nc.scalar.activation(
                                    hw_[:], hw_[:],
                                    mybir.ActivationFunctionType.Copy,
                                    scale=dinvp[:, w:w + 1])
                            nc.vector.tensor_tensor(
                                out=hw_[:], in0=hw_[:], in1=bias_t[:], op=AG.add)
                            if relu:
                                nc.scalar.activation(
                                    hw_[:], hw_[:],
                                    mybir.ActivationFunctionType.Relu)
                            if hT_next is not None:
                                pt = psR.tile([64, 128], F32, tag="ptr")
                                nc.tensor.transpose(pt[:, :nw], hw_[:nw, :],
                                                    id128[:nw, :nw])
                                j, r0 = c0 // 512, c0 % 512
                                if CP & 4:
                                    nc.vector.tensor_copy(
                                        hT_next[j][:, r0:r0 + nw], pt[:, :nw])
                                else:
                                    nc.scalar.copy(hT_next[j][:, r0:r0 + nw],
                                                   pt[:, :nw])
                            else:
                                nc.sync.dma_start(out_d[c0:c0 + nw, :],
                                                  hw_[:nw, :])

                seg_pass(False)
                if split:
                    seg_pass(True)

            PH = int(os.environ.get("K_PHASES", "9"))
            ONLY_AGG = os.environ.get("K_ONLY_AGG", "0") == "1"
            if ONLY_AGG:
                for _l in range(3):
                    aggregate(0, b1t, True, hT2)
                PH = 0
            REP = int(os.environ.get("K_REPEAT", "1"))
            for _rep in range(REP):
                if PH >= 1:
                    TT = transform(0, None, 128, W1t)
                if PH >= 2:
                    build_table(0, TT)
                if PH >= 3:
                    aggregate(0, b1t, True, hT2)
                if PH >= 4:
                    TT = transform(1, hT2, 64, W2t)
                if PH >= 5:
                    build_table(1, TT)
                if PH >= 6:
                    aggregate(1, b2t, True, hT3)
                if PH >= 7:
                    TT = transform(2, hT3, 64, W34t)
                if PH >= 8:
                    build_table(2, TT)
                if PH >= 9:
                    aggregate(2, b34t, False, None)

    nc.compile()
    return nc


def _run(inputs, N, E):
    NLOC = N // N_CORES
    W = (NLOC + 127) // 128

    x = np.asarray(inputs["x"], np.float32)
    ei = np.asarray(inputs["edge_index"], np.int64)
    W1 = np.asarray(inputs["W1"], np.float32)
    b1 = np.asarray(inputs["b1"], np.float32)
    W2 = np.asarray(inputs["W2"], np.float32)
    b2 = np.asarray(inputs["b2"], np.float32)
    Wmu = np.asarray(inputs["Wmu"], np.float32)
    bmu = np.asarray(inputs["bmu"], np.float32)
    Wls = np.asarray(inputs["Wls"], np.float32)
    bls = np.asarray(inputs["bls"], np.float32)

    loop = np.arange(N, dtype=np.int64)
    src = np.concatenate([ei[0], loop])
    dst = np.concatenate([ei[1], loop])
    deg = np.bincount(dst, minlength=N).astype(np.float32)

    # balanced window assignment (equalizes per-window counts across cores)
    lpos = _balance(deg, N, NLOC, W)

    idx_tiles, dsel_tiles, P_ws, P_tot = _prep_edges(src, dst, N, NLOC, W, lpos)

    key = (N, NLOC, W, tuple(P_ws[0]), tuple(P_ws[1]),
           os.environ.get("K_PHASES", "9"), os.environ.get("K_REPEAT", "1"),
           os.environ.get("K_SOLO", "0"), os.environ.get("K_ONLY_AGG", "0"),
           os.environ.get("K_CP", "0"), os.environ.get("K_GRP", "4"))
    if key not in _cache:
        _cache[key] = _build(N, NLOC, W, P_ws)
    nc = _cache[key]

    W34 = np.concatenate([Wmu, Wls], axis=1)
    b34 = np.concatenate([bmu, bls])
    iota = np.ascontiguousarray(np.tile(np.arange(256, dtype=np.float32),
                                        (128, 1))).astype(BF16)
    id128 = np.eye(128, dtype=np.float32)
    id64 = np.eye(64, dtype=np.float32).astype(BF16)
    b1b = np.ascontiguousarray(np.tile(b1, (128, 1)))
    b2b = np.ascontiguousarray(np.tile(b2, (128, 1)))
    b34b = np.ascontiguousarray(np.tile(b34, (128, 1)))

    in_maps = []
    for c in range(N_CORES):
        degc = deg[c * NLOC:(c + 1) * NLOC]
        lc = lpos[c * NLOC:(c + 1) * NLOC]
        degp = np.ones(W * 128, np.float32)
        degp[lc] = degc
        xp = np.empty((NLOC, x.shape[1]), np.float32)
        xp[lc] = x[c * NLOC:(c + 1) * NLOC]
        in_maps.append({
            "xT": np.ascontiguousarray(xp.T),
            "idxs": idx_tiles[c],
            "dsel": dsel_tiles[c],
            "degp": np.ascontiguousarray(degp.reshape(W, 128).T),
            "W1": W1, "W2": W2, "W34": W34,
            "b1b": b1b, "b2b": b2b, "b34b": b34b,
            "iota": iota, "id128": id128, "id64": id64,
        })

    _last["nc"] = nc
    _last["in_maps"] = in_maps
    res = run_bass_kernel_spmd(nc, in_maps, core_ids=list(range(N_CORES)))
    out = np.empty((N, 64), np.float32)
    for c in range(N_CORES):
        lc = lpos[c * NLOC:(c + 1) * NLOC]
        out[c * NLOC:(c + 1) * NLOC] = res.results[c]["out34"][lc]
    return out[:, :32].copy(), out[:, 32:].copy()


def kernel(**inputs):
    x = np.asarray(inputs["x"])
    ei = np.asarray(inputs["edge_index"])
    return _run(inputs, x.shape[0], ei.shape[1])



# revision 7
# speedup vs baseline: 1.0002x; 1.0002x over previous
"""3-layer GCN encoder (GCNConv x4, layers 3+4 fused) on 8 Trainium2 NeuronCores.

Strategy v2 (graph/data parallel, edge-cut per the sharding hint):
  - Nodes partitioned across 8 cores; per layer the local transform
    H = h @ W runs on PE, rows are scaled by dinv[node], cast to bf16 and
    packed two-nodes-per-256B-row into a table slice that is AllGathered.
  - Pair-packing v2: a host-side greedy matching pairs nodes whose out-edges
    hit the same dst windows, so one 256B fetch often serves TWO edges. The
    one-hot selector S routes the two packed nodes independently (dual dsel
    streams) on tiles that contain such slots; plain tiles keep the single
    256-wide is_equal.
  - Self-loops never touch DMA: dinv^2 * h is added into PSUM with one
    identity matmul per window directly from the on-chip table staging tile.
  - Single table chunk (25600 pair rows < int16 range), so no A/B split.
  - Aggregation: per 128-dst window, dma_gather fetches pair rows for all
    in-edge slots (dst-grouped, padded to x128); S turns segment-sum into PE
    matmuls accumulating in PSUM; epilogue applies dinv[dst], bias, ReLU.

Self-contained: only needs numpy/ml_dtypes/concourse (container-installed).
"""

import os
import sys

if "/opt/trn_rl_repo" not in sys.path:
    sys.path.insert(0, "/opt/trn_rl_repo")

import numpy as np
import ml_dtypes

import concourse.bass as bass
import concourse.bacc as bacc
import concourse.mybir as mybir
import concourse.tile as tile
from concourse.bass_utils import run_bass_kernel_spmd

BF16 = ml_dtypes.bfloat16
F32 = mybir.dt.float32
BF = mybir.dt.bfloat16
I16 = mybir.dt.int16

N_CORES = 8

_cache = {}
_last = {}


def last_run(trace=False, **kw):
    """Re-run the last compiled kernel/in_maps (optionally with NTFF tracing)."""
    if "nc" not in _last:
        return None
    return run_bass_kernel_spmd(_last["nc"], _last["in_maps"],
                                core_ids=list(range(N_CORES)), trace=trace, **kw)


def _match_place(src, dst, N, NLOC, W):
    """Degree-balanced window assignment + within-window pairing.

    Windows are fixed up front (deal nodes by in-degree round-robin over
    all N_CORES*W windows, respecting per-window capacity), then each
    window's members are greedily matched by shared-out-window overlap.
    A matched pair sits at adjacent rels (2k, 2k+1) of its window, sharing
    one 256B table row. Returns (ncore, lpos)."""
    NW = N_CORES * W
    deg = np.bincount(dst, minlength=N)
    order = np.argsort(-deg, kind="stable")
    cap = np.full(NW, 128, np.int64)
    last = NLOC - 128 * (W - 1)
    cap[(W - 1) * N_CORES:] = last  # global windows (W-1)*8.. are the short ones
    # global window id g: core = g % N_CORES, local window = g // N_CORES
    win_of = np.empty(N, np.int64)
    fill = np.zeros(NW, np.int64)
    gi = 0
    for n in order:
        while fill[gi % NW] >= cap[gi % NW]:
            gi += 1
        g = gi % NW
        win_of[n] = g
        fill[g] += 1
        gi += 1

    # (src, dst-window) incidence with multiplicity
    key = src * NW + win_of[dst]
    uk, cnt = np.unique(key, return_counts=True)
    u_s = (uk // NW).astype(np.int64)
    u_w = (uk % NW).astype(np.int64)

    mem_of = [np.nonzero(win_of == g)[0] for g in range(NW)]
    idx_in = np.empty(N, np.int64)
    for g in range(NW):
        idx_in[mem_of[g]] = np.arange(len(mem_of[g]))

    pool_of_inc = win_of[u_s]
    ncore = np.empty(N, np.int64)
    lpos = np.empty(N, np.int64)
    for g in range(NW):
        mem = mem_of[g]
        P = len(mem)
        m = np.nonzero(pool_of_inc == g)[0]
        S = np.zeros((P, P), np.int32)
        ow = u_w[m]
        ns = idx_in[u_s[m]]
        c = cnt[m]
        o = np.argsort(ow, kind="stable")
        ow, ns, c = ow[o], ns[o], c[o]
        b = np.nonzero(np.diff(ow))[0] + 1
        for seg in np.split(np.arange(len(ow)), b):
            if len(seg) < 2:
                continue
            nn = ns[seg]
            cc = c[seg]
            S[np.ix_(nn, nn)] += np.minimum.outer(cc, cc)
        np.fill_diagonal(S, 0)
        iu, iv = np.triu_indices(P, 1)
        sc = S[iu, iv]
        o2 = np.argsort(-sc, kind="stable")
        used = np.zeros(P, bool)
        pairs = []
        nmatched = 0
        for k in o2:
            if sc[k] <= 0:
                break
            a, b2 = iu[k], iv[k]
            if used[a] or used[b2]:
                continue
            used[a] = used[b2] = True
            pairs.append((a, b2))
            nmatched += 2
            if nmatched >= P - 1:
                break
        lf = np.nonzero(~used)[0]
        for a, b2 in zip(lf[0::2], lf[1::2]):
            pairs.append((a, b2))
        core = g % N_CORES
        wl = g // N_CORES
        for k, (a, b2) in enumerate(pairs):
            ncore[mem[a]] = ncore[mem[b2]] = core
            lpos[mem[a]] = wl * 128 + 2 * k
            lpos[mem[b2]] = wl * 128 + 2 * k + 1
    return ncore, lpos


def _prep_edges(src, dst, ncore, lpos, N, NLOC, W):
    """Group in-edges by (dst core, dst window, src pair-row); one slot per
    max(parity0,parity1) count. Dual-routed slots sorted first per window.

    Returns per-core idx/dsel tiles plus shared (NT_w, ND_w) tile counts."""
    Wp = W + (W & 1)
    RLOC = 64 * Wp                  # pair rows per core (3200)

    sc = ncore[src]
    sl = lpos[src]
    rel_s = sl & 127
    ws = sl >> 7
    srow_all = sc * RLOC + ws * 64 + (rel_s >> 1)
    par_all = rel_s & 1

    dc = ncore[dst]
    dl = lpos[dst]
    w_all = dl >> 7
    rel_all = dl & 127

    per_core = []
    ND = np.zeros(W, np.int64)
    NT_solo = np.zeros(W, np.int64)
    for c in range(N_CORES):
        m = dc == c
        w = w_all[m]
        rel = rel_all[m]
        srow = srow_all[m]
        par = par_all[m]
        # group id per (w, srow)
        gkey = w * (N_CORES * RLOC) + srow
        ug, gid = np.unique(gkey, return_inverse=True)
        g_w = ug // (N_CORES * RLOC)
        g_row = ug % (N_CORES * RLOC)
        nG = len(ug)
        c01 = np.zeros((nG, 2), np.int64)
        np.add.at(c01, (gid, par), 1)
        gmax = c01.max(1)
        gmin = c01.min(1)
        # j = position within (group, parity)
        order = np.lexsort((par, gid))
        j = np.empty(len(w), np.int64)
        gp = gid[order] * 2 + par[order]
        gstart = np.concatenate([[0], np.cumsum(np.bincount(
            gp, minlength=2 * nG))])
        j[order] = np.arange(len(w)) - gstart[gp]
        # per-window dual/solo slot counts
        ndual_w = np.zeros(W, np.int64)
        np.add.at(ndual_w, g_w, gmin)
        ntot_w = np.zeros(W, np.int64)
        np.add.at(ntot_w, g_w, gmax)
        # slot index within group = j; slot is dual if j < gmin
        per_core.append(dict(w=w, rel=rel, par=par, gid=gid, j=j,
                             g_w=g_w, g_row=g_row, gmax=gmax, gmin=gmin,
                             ndual_w=ndual_w, ntot_w=ntot_w))
        ND = np.maximum(ND, (ndual_w + 127) // 128)
        NT_solo = np.maximum(NT_solo, (ntot_w + 127) // 128)
    NT = np.maximum(ND, NT_solo)
    base = np.concatenate([[0], np.cumsum(NT * 128)])
    P_tot = int(base[-1])

    idx_tiles, dsel_tiles = [], []
    for c in range(N_CORES):
        pc = per_core[c]
        # order groups: within each window, dual slots first. A slot (g, j)
        # goes to window-local position: dual (j < gmin): dualbase[g] + j
        # where dual slots of groups are packed in group order; solo:
        # ndual_w + solobase[g] + (j - gmin).
        g_w, gmax, gmin = pc["g_w"], pc["gmax"], pc["gmin"]
        nG = len(g_w)
        worder = np.argsort(g_w, kind="stable")
        dual_cum = np.zeros(nG, np.int64)
        solo_cum = np.zeros(nG, np.int64)
        ndual_w, ntot_w = pc["ndual_w"], pc["ntot_w"]
        wstart_d = np.concatenate([[0], np.cumsum(ndual_w)])
        wstart_s = np.concatenate([[0], np.cumsum(ntot_w - ndual_w)])
        dual_cum[worder] = (np.cumsum(gmin[worder]) - gmin[worder]
                            - wstart_d[g_w[worder]])
        nsolo_g = gmax - gmin
        solo_cum[worder] = (np.cumsum(nsolo_g[worder]) - nsolo_g[worder]
                            - wstart_s[g_w[worder]])

        w, rel, par, gid, j = pc["w"], pc["rel"], pc["par"], pc["gid"], pc["j"]
        isd = j < gmin[gid]
        local = np.where(
            isd, dual_cum[gid] + j,
            ndual_w[w] + solo_cum[gid] + (j - gmin[gid]))
        slot = base[w] + local

        # slot attribute arrays
        idx_arr = np.zeros(P_tot, np.int16)
        d0 = np.full(P_tot, 300.0, np.float32)
        d1 = np.full(P_tot, 300.0, np.float32)
        # dual region encoding: stream by parity; combined region: 0..255 val
        tile_of = local >> 7
        in_dual_tile = tile_of < ND[w]
        gs = np.nonzero(np.ones_like(slot, bool))[0]
        # each group's slot row: same srow for all j
        srow_slot = pc["g_row"][gid].astype(np.int16)
        idx_arr[slot] = srow_slot
        enc_dual0 = rel.astype(np.float32)
        enc_dual1 = 128.0 + rel.astype(np.float32)
        comb = rel + 128.0 * par
        m0 = in_dual_tile & (par == 0)
        m1 = in_dual_tile & (par == 1)
        ms = ~in_dual_tile
        d0[slot[m0]] = enc_dual0[m0]
        d1[slot[m1]] = enc_dual1[m1]
        d0[slot[ms]] = comb[ms]
        del gs

        idx16 = np.ascontiguousarray(idx_arr.reshape(P_tot // 16, 16).T)
        idx_tiles.append(np.ascontiguousarray(np.tile(idx16, (8, 1))))
        nt_tot = P_tot // 128
        dI = np.empty((128, 2 * nt_tot), np.float32)
        dI[:, 0::2] = d0.reshape(nt_tot, 128).T
        dI[:, 1::2] = d1.reshape(nt_tot, 128).T
        dsel_tiles.append(np.ascontiguousarray(dI))
    return idx_tiles, dsel_tiles, (list(map(int, NT)), list(map(int, ND))), P_tot


def _build(N, NLOC, W, NT_ND):
    """Build the 8-core SPMD Bass program. Returns compiled nc."""
    NT, ND = [list(map(int, x)) for x in NT_ND]
    NT_MAX = max(NT)
    P_tot = 128 * sum(NT)
    cums = np.concatenate([[0], np.cumsum(np.array(NT) * 128)]).astype(int)
    GRP = int(os.environ.get("K_GRP", "7"))

    solo = os.environ.get("K_SOLO", "0") == "1"
    nc = bacc.Bacc("TRN2", target_bir_lowering=False, debug=False,
                   num_devices=1 if solo else N_CORES)

    Wp = W + (W & 1)
    RLOC = 64 * Wp

    xT_d = nc.dram_tensor("xT", (128, NLOC), F32, kind="ExternalInput")
    idxs_d = nc.dram_tensor("idxs", (128, P_tot // 16), I16, kind="ExternalInput")
    dsel_d = nc.dram_tensor("dsel", (128, 2 * (P_tot // 128)), F32,
                            kind="ExternalInput")
    degp_d = nc.dram_tensor("degp", (128, W), F32, kind="ExternalInput")
    W1_d = nc.dram_tensor("W1", (128, 64), F32, kind="ExternalInput")
    W2_d = nc.dram_tensor("W2", (64, 64), F32, kind="ExternalInput")
    W34_d = nc.dram_tensor("W34", (64, 64), F32, kind="ExternalInput")
    b1_d = nc.dram_tensor("b1b", (128, 64), F32, kind="ExternalInput")
    b2_d = nc.dram_tensor("b2b", (128, 64), F32, kind="ExternalInput")
    b34_d = nc.dram_tensor("b34b", (128, 64), F32, kind="ExternalInput")
    iota_d = nc.dram_tensor("iota", (128, 256), BF, kind="ExternalInput")
    id128_d = nc.dram_tensor("id128", (128, 128), F32, kind="ExternalInput")
    id128b_d = nc.dram_tensor("id128b", (128, 128), BF, kind="ExternalInput")
    id64_d = nc.dram_tensor("id64", (64, 64), BF, kind="ExternalInput")
    out_d = nc.dram_tensor("out34", (NLOC, 64), F32, kind="ExternalOutput")

    tabl = [nc.dram_tensor(f"tabl{l}", (RLOC, 128), BF, kind="Internal")
            for l in range(3)]
    tabf = [nc.dram_tensor(f"tabf{l}", (N_CORES * RLOC, 128), BF,
                           kind="Internal", addr_space="Shared")
            for l in range(3)]

    AG = mybir.AluOpType
    RG = [list(range(N_CORES))]

    with tile.TileContext(nc) as tc:
        with (
            tc.tile_pool(name="const", bufs=1) as const,
            tc.tile_pool(name="big", bufs=1) as big,
            tc.tile_pool(name="tt", bufs=int(os.environ.get("K_TT", "2"))) as ttp,
            tc.tile_pool(name="work", bufs=int(os.environ.get("K_WK", "8"))) as work,
            tc.tile_pool(name="gp", bufs=int(os.environ.get("K_GP", "2"))) as gp,
            tc.tile_pool(name="sp", bufs=int(os.environ.get("K_SB", "8"))) as sp,
            tc.tile_pool(name="psT", bufs=2, space="PSUM") as psT,
            tc.tile_pool(name="psR", bufs=2, space="PSUM") as psR,
            tc.tile_pool(name="psA", bufs=int(os.environ.get("K_PSA", "4")),
                         space="PSUM") as psA,
        ):
            def cload(dram, shape, dt, tag):
                t = const.tile(shape, dt, tag=tag)
                nc.sync.dma_start(t[:], dram[:])
                return t

            idxs = cload(idxs_d, [128, P_tot // 16], I16, "idxs")
            dsel = cload(dsel_d, [128, 2 * (P_tot // 128)], F32, "dsel")
            iota = cload(iota_d, [128, 256], BF, "iota")
            id128 = cload(id128_d, [128, 128], F32, "id128")
            id128b = cload(id128b_d, [128, 128], BF, "id128b")
            id64 = cload(id64_d, [64, 64], BF, "id64")
            W1t = cload(W1_d, [128, 64], F32, "W1t")
            W2t = cload(W2_d, [64, 64], F32, "W2t")
            W34t = cload(W34_d, [64, 64], F32, "W34t")
            b1t = cload(b1_d, [128, 64], F32, "b1t")
            b2t = cload(b2_d, [128, 64], F32, "b2t")
            b34t = cload(b34_d, [128, 64], F32, "b34t")

            NCH = (NLOC + 511) // 512
            hT2 = [big.tile([64, 512], F32, name=f"hT2_{j}", tag=f"hT2_{j}")
                   for j in range(NCH)]
            hT3 = [big.tile([64, 512], F32, name=f"hT3_{j}", tag=f"hT3_{j}")
                   for j in range(NCH)]

            degp = const.tile([128, W], F32, tag="degp")
            nc.sync.dma_start(degp[:], degp_d[:])
            sqp = const.tile([128, W], F32, tag="sqp")
            nc.scalar.activation(sqp[:], degp[:], mybir.ActivationFunctionType.Sqrt)
            dinvp = const.tile([128, W], F32, tag="dinvp")
            nc.vector.reciprocal(dinvp[:], sqp[:])

            def transform(l, hT, K, Wt):
                """T^T = W^T @ hT, bf16 [64, NLOC] (dinv applied in build_table)."""
                TT = ttp.tile([64, NLOC], BF, tag="TT")
                for c0 in range(0, NLOC, 512):
                    cn = min(512, NLOC - c0)
                    if hT is None:
                        xc = work.tile([128, 512], F32, tag="xc")
                        nc.sync.dma_start(xc[:, :cn], xT_d[:, c0:c0 + cn])
                        rhs = xc[:K, :cn]
                    else:
                        rhs = hT[c0 // 512][:K, :cn]
                    ps = psT.tile([64, 512], F32, tag="psT")
                    nc.tensor.matmul(ps[:, :cn], Wt[:K, :], rhs,
                                     start=True, stop=True)
                    nc.scalar.copy(TT[:, c0:c0 + cn], ps[:, :cn])
                return TT

            def build_table(l, TT):
                stage = ttp.tile([128, Wp * 64], BF, tag="stage")
                if W != Wp:
                    nc.vector.memset(stage[:, W * 64:], 0.0)
                for w in range(W):
                    c0 = 128 * w
                    nw = min(128, NLOC - c0)
                    ptt = psR.tile([128, 128], BF, tag="ptr")
                    pt = ptt[:, :64]
                    nc.tensor.transpose(pt[:nw, :], TT[:, c0:c0 + nw], id64[:, :])
                    if nw < 128:
                        nc.vector.memset(stage[:, 64 * w:64 * (w + 1)], 0.0)
                    nc.scalar.activation(
                        stage[:nw, 64 * w:64 * w + 64], pt[:nw, :],
                        mybir.ActivationFunctionType.Copy,
                        scale=dinvp[:nw, w:w + 1])
                nc.sync.dma_start(
                    tabl[l][:].rearrange("(w q) (r e) -> (q r) w e",
                                         w=Wp, q=64, r=2),
                    stage[:].rearrange("p (w e) -> p w e", w=Wp))
                if not solo:
                    nc.gpsimd.collective_compute(
                        "AllGather", AG.bypass, replica_groups=RG,
                        ins=[tabl[l][:].opt()], outs=[tabf[l][:].opt()])
                return stage

            def aggregate(l, stage, bias_t, relu, hT_next):
                for w0 in range(0, W, GRP):
                    wn = min(GRP, W - w0)
                    ntg = sum(NT[w0:w0 + wn])
                    pg = int(cums[w0 + wn] - cums[w0])
                    soff = int(cums[w0]) // 16
                    g = gp.tile([128, GRP * NT_MAX, 128], BF, tag="g")
                    nc.gpsimd.dma_gather(
                        g[:, :ntg, :], tabf[l][:],
                        idxs[:, soff: soff + pg // 16],
                        pg, pg, 128, single_packet=False)
                    tb = 0
                    for w in range(w0, w0 + wn):
                        c0 = 128 * w
                        nw = min(128, NLOC - c0)
                        nt = NT[w]
                        nd = ND[w]
                        toff = int(cums[w]) // 128
                        ps = psA.tile([128, 64], F32, tag="psA")
                        for t in range(nt):
                            S = sp.tile([128, 256], BF, tag="S")
                            dc = 2 * (toff + t)
                            if t < nd:
                                nc.vector.tensor_scalar(
                                    out=S[:, 0:128], in0=iota[:, 0:128],
                                    scalar1=dsel[:, dc:dc + 1],
                                    scalar2=None, op0=AG.is_equal)
                                nc.vector.tensor_scalar(
                                    out=S[:, 128:256], in0=iota[:, 128:256],
                                    scalar1=dsel[:, dc + 1:dc + 2],
                                    scalar2=None, op0=AG.is_equal)
                            else:
                                nc.vector.tensor_scalar(
                                    out=S[:], in0=iota[:],
                                    scalar1=dsel[:, dc:dc + 1],
                                    scalar2=None, op0=AG.is_equal)
                            nc.tensor.matmul(
                                ps[:], S[:, 0:128], g[:, tb + t, 0:64],
                                start=(t == 0), stop=False)
                            nc.tensor.matmul(
                                ps[:], S[:, 128:256], g[:, tb + t, 64:128],
                                start=False, stop=False)
                        tb += nt
                        # self-loop: ps += stage_w (= dinv * T, node-major)
                        nc.tensor.matmul(
                            ps[:], id128b[:, :],
                            stage[:, 64 * w:64 * w + 64],
                            start=False, stop=True)
                        hw_ = work.tile([128, 64], F32, tag="hw")
                        nc.scalar.activation(
                            hw_[:], ps[:],
                            mybir.ActivationFunctionType.Copy,
                            scale=dinvp[:, w:w + 1])
                        nc.vector.tensor_tensor(
                            out=hw_[:], in0=hw_[:], in1=bias_t[:], op=AG.add)
                        if relu:
                            nc.scalar.activation(
                                hw_[:], hw_[:],
                                mybir.ActivationFunctionType.Relu)
                        if hT_next is not None:
                            pt = psR.tile([64, 128], F32, tag="ptr")
                            nc.tensor.transpose(pt[:, :nw], hw_[:nw, :],
                                                id128[:nw, :nw])
                            jj, r0 = c0 // 512, c0 % 512
                            nc.scalar.copy(hT_next[jj][:, r0:r0 + nw],
                                           pt[:, :nw])
                        else:
                            nc.sync.dma_start(out_d[c0:c0 + nw, :],
                                              hw_[:nw, :])

            PH = int(os.environ.get("K_PHASES", "9"))
            REP = int(os.environ.get("K_REPEAT", "1"))
            for _rep in range(REP):
                if PH >= 1:
                    TT = transform(0, None, 128, W1t)
                if PH >= 2:
                    stage = build_table(0, TT)
                if PH >= 3:
                    aggregate(0, stage, b1t, True, hT2)
                if PH >= 4:
                    TT = transform(1, hT2, 64, W2t)
                if PH >= 5:
                    stage = build_table(1, TT)
                if PH >= 6:
                    aggregate(1, stage, b2t, True, hT3)
                if PH >= 7:
                    TT = transform(2, hT3, 64, W34t)
                if PH >= 8:
                    stage = build_table(2, TT)
                if PH >= 9:
                    aggregate(2, stage, b34t, False, None)

    nc.compile()
    return nc


def _prep_all(inputs):
    """Host-side prep: pairing, placement, edge grouping, per-core inputs.
    Returns (nc_key_parts, per-core in_maps, ncore, lpos, N, NLOC, W)."""
    x = np.asarray(inputs["x"], np.float32)
    ei = np.asarray(inputs["edge_index"], np.int64)
    N = x.shape[0]
    NLOC = N // N_CORES
    W = (NLOC + 127) // 128

    src = ei[0].copy()
    dst = ei[1].copy()
    deg = np.bincount(dst, minlength=N).astype(np.float32) + 1.0  # + self-loop

    ncore, lpos = _match_place(src, dst, N, NLOC, W)
    idx_tiles, dsel_tiles, NT_ND, P_tot = _prep_edges(
        src, dst, ncore, lpos, N, NLOC, W)
    return dict(N=N, NLOC=NLOC, W=W, NT_ND=NT_ND, P_tot=P_tot,
                ncore=ncore, lpos=lpos, deg=deg,
                idx_tiles=idx_tiles, dsel_tiles=dsel_tiles)


def _run(inputs):
    prep = _prep_all(inputs)
    N, NLOC, W = prep["N"], prep["NLOC"], prep["W"]
    NT, ND = prep["NT_ND"]
    ncore, lpos, deg = prep["ncore"], prep["lpos"], prep["deg"]

    x = np.asarray(inputs["x"], np.float32)
    W1 = np.asarray(inputs["W1"], np.float32)
    b1 = np.asarray(inputs["b1"], np.float32)
    W2 = np.asarray(inputs["W2"], np.float32)
    b2 = np.asarray(inputs["b2"], np.float32)
    Wmu = np.asarray(inputs["Wmu"], np.float32)
    bmu = np.asarray(inputs["bmu"], np.float32)
    Wls = np.asarray(inputs["Wls"], np.float32)
    bls = np.asarray(inputs["bls"], np.float32)

    key = (N, NLOC, W, tuple(NT), tuple(ND),
           os.environ.get("K_PHASES", "9"), os.environ.get("K_REPEAT", "1"),
           os.environ.get("K_SOLO", "0"), os.environ.get("K_GRP", "7"))
    if key not in _cache:
        _cache[key] = _build(N, NLOC, W, (NT, ND))
    nc = _cache[key]

    W34 = np.concatenate([Wmu, Wls], axis=1)
    b34 = np.concatenate([bmu, bls])
    iota = np.ascontiguousarray(np.tile(np.arange(256, dtype=np.float32),
                                        (128, 1))).astype(BF16)
    id128 = np.eye(128, dtype=np.float32)
    id128b = np.eye(128, dtype=np.float32).astype(BF16)
    id64 = np.eye(64, dtype=np.float32).astype(BF16)
    b1b = np.ascontiguousarray(np.tile(b1, (128, 1)))
    b2b = np.ascontiguousarray(np.tile(b2, (128, 1)))
    b34b = np.ascontiguousarray(np.tile(b34, (128, 1)))

    in_maps = []
    for c in range(N_CORES):
        nodes = np.nonzero(ncore == c)[0]
        lc = lpos[nodes]
        degp = np.ones(W * 128, np.float32)
        degp[lc] = deg[nodes]
        xp = np.empty((NLOC, x.shape[1]), np.float32)
        xp[lc] = x[nodes]
        in_maps.append({
            "xT": np.ascontiguousarray(xp.T),
            "idxs": prep["idx_tiles"][c],
            "dsel": prep["dsel_tiles"][c],
            "degp": np.ascontiguousarray(degp.reshape(W, 128).T),
            "W1": W1, "W2": W2, "W34": W34,
            "b1b": b1b, "b2b": b2b, "b34b": b34b,
            "iota": iota, "id128": id128, "id128b": id128b, "id64": id64,
        })

    _last["nc"] = nc
    _last["in_maps"] = in_maps
    res = run_bass_kernel_spmd(nc, in_maps, core_ids=list(range(N_CORES)))
    out = np.empty((N, 64), np.float32)
    for c in range(N_CORES):
        nodes = np.nonzero(ncore == c)[0]
        lc = lpos[nodes]
        out[nodes] = res.results[c]["out34"][lc]
    return out[:, :32].copy(), out[:, 32:].copy()


def kernel(**inputs):
    return _run(inputs)
